# revision 19
# baseline (speedup 1.0000x reference)
"""AnomalyTransformer Trainium2 kernel: 8-core SPMD, sequence-parallel over L.

Core i owns L-rows [64i, 64i+64) for all batches. Per layer each core computes
K/V for its own token rows and the cores AllGather them (bf16, 512KB/rank);
scores, softmax(series), Gaussian prior, sigma, attention output, FFN and
LayerNorms run on owned rows only. The tiny front-end enhancement is
replicated on every core; per-core "window" inputs carry the circularly /
edge-padded x slices so all cores run one identical program (SPMD).
"""

import contextlib
import math
import sys

sys.path.insert(0, "/opt/trn_rl_repo")

import numpy as np
import ml_dtypes

import concourse.bass as bass
import concourse.tile as tile
from concourse import mybir, bacc
from concourse.masks import make_identity

F32 = mybir.dt.float32
BF16 = mybir.dt.bfloat16
AX = mybir.AxisListType
ALU = mybir.AluOpType
ACTF = mybir.ActivationFunctionType

NCORES = 8
B, L, C = 4, 512, 55
D, H, DH, DFF, NL = 512, 8, 64, 512, 3
LS = L // NCORES          # 64 own rows per core
TOK = B * LS              # 256 own tokens
TC = TOK // 128           # 2 token chunks
DC = D // 128             # 4 D chunks
HID = 16
SCALE = 1.0 / math.sqrt(DH)
LN3 = math.log(3.0)
INV_SQRT2PI = 1.0 / math.sqrt(2.0 * math.pi)

BF = ml_dtypes.bfloat16


def _declare_params(nc):
    p = {}

    def di(name, shape, dt=F32):
        p[name] = nc.declare_dram_parameter(name, list(shape), dt, isOutput=False)

    # per-core varying
    di("xw5T", (B, 5, C, LS + 2))
    di("d2dup", (128, L))
    di("pedup", (128, D))
    # replicated
    di("xT", (B, C, L))
    di("node_embT", (HID, C))
    di("eye55", (C, C))
    for n, shape in [
        ("lfc1_w", (4, HID)), ("lfc2_w", (HID, HID)), ("gq_w", (4, HID)),
        ("gk_w", (4, HID)), ("gfc_w", (4, HID)), ("ffc_w", (HID, HID)),
        ("ofc_w", (HID, 1)),
    ]:
        di(n, shape)
    for n in ["lfc1_b", "lfc2_b", "gq_b", "gk_b", "gfc_b", "ffc_b",
              "hn_g", "hn_b"]:
        di(n, (HID,))
    di("ofc_b", (1,))
    di("tokw", (192, D))          # f32, taps at 64-row pitch, zero padded
    for i in range(NL):
        for n in ["wq", "wk", "wv", "wo", "c1", "c2"]:
            di(f"{n}{i}", (D, D), BF16)
        di(f"wsig{i}", (D, H), BF16)
        for n in ["bq", "bk", "bv", "bo", "c1b", "c2b"]:
            di(f"{n}{i}", (1, D), BF16)
        di(f"bsig{i}", (1, H), BF16)
        for n in ["ln1g", "ln1b", "ln2g", "ln2b"]:
            di(f"{n}{i}", (D,))
    di("lnfg", (D,))
    di("lnfb", (D,))
    di("projw", (D, C), BF16)
    di("projb", (1, C), BF16)

    outs = {
        "o_dbg": nc.declare_dram_parameter("o_dbg", [8, 128, 512], F32, isOutput=True),
        "o_series": nc.declare_dram_parameter("o_series", [NL, H, TC, 128, L], F32, isOutput=True),
        "o_prior": nc.declare_dram_parameter("o_prior", [NL, H, TC, 128, L], F32, isOutput=True),
        "o_sigma": nc.declare_dram_parameter("o_sigma", [NL, H, TC, 128, L], F32, isOutput=True),
        "o_final": nc.declare_dram_parameter("o_final", [TC, 128, C], F32, isOutput=True),
    }
    return p, outs


def build():
    nc = bacc.Bacc()
    prm, outs = _declare_params(nc)
    agin = nc.dram_tensor("agin", [1024, 256], BF16)
    agout = nc.dram_tensor("agout", [8192, 256], BF16, addr_space="Shared")
    wuin = nc.dram_tensor("wuin", [1, 64], F32)
    wuout = nc.dram_tensor("wuout", [8, 64], F32, addr_space="Shared")
    with tile.TileContext(nc) as tc_:
        _body(nc, tc_, prm, outs, agin, agout, wuin, wuout)
    nc.finalize()
    return nc


def _body(nc, tc_, prm, outs, agin, agout, wuin, wuout):
    ctx = contextlib.ExitStack()
    with ctx:
        cpool = ctx.enter_context(tc_.tile_pool(name="cpool", bufs=1))
        wpool = ctx.enter_context(tc_.tile_pool(name="wpool", bufs=2))
        lnpool = ctx.enter_context(tc_.tile_pool(name="lnpool", bufs=1))
        xpool = ctx.enter_context(tc_.tile_pool(name="xpool", bufs=2))
        s2pool = ctx.enter_context(tc_.tile_pool(name="s2pool", bufs=1))
        kvpool = ctx.enter_context(tc_.tile_pool(name="kvpool", bufs=1))
        qpool = ctx.enter_context(tc_.tile_pool(name="qpool", bufs=3))
        smpool = ctx.enter_context(tc_.tile_pool(name="smpool", bufs=6))
        ppool = ctx.enter_context(tc_.tile_pool(name="ppool", bufs=2, space="PSUM"))

        # warm up the collective machinery while the front-end runs
        wut = cpool.tile([1, 64], F32)
        nc.vector.memset(wut, 0.0)
        nc.sync.dma_start(out=wuin[:, :], in_=wut)
        nc.gpsimd.collective_compute(
            "AllGather", ALU.bypass, replica_groups=[list(range(NCORES))],
            ins=[wuin[:, :]], outs=[wuout[:, :]],
        )

        # ---- constants ----
        idf = cpool.tile([128, 128], F32)
        make_identity(nc, idf)
        idb = cpool.tile([128, 128], BF16)
        make_identity(nc, idb)
        ones_bf = cpool.tile([1, 512], BF16)
        nc.vector.memset(ones_bf, 1.0)
        eps = cpool.tile([128, 1], F32)
        nc.vector.memset(eps, 1e-5)
        b3eps = cpool.tile([128, 1], F32)
        nc.vector.memset(b3eps, LN3 * 1e-5)
        d2dup = cpool.tile([128, L], F32)
        nc.sync.dma_start(out=d2dup, in_=prm["d2dup"][:, :])
        pedup = cpool.tile([128, D], F32)
        nc.sync.dma_start(out=pedup, in_=prm["pedup"][:, :])

        def bc_tile(pool, name, width, dt=F32, tag=None):
            t = pool.tile([128, width], dt, tag=tag or f"bc_{name}")
            src = prm[name].ap()
            bcast = bass.AP(tensor=src.tensor, offset=src.offset,
                            ap=[[0, 128]] + src.ap)
            nc.sync.dma_start(out=t, in_=bcast)
            return t

        def ln_inplace(x_sl, g_bc, b_bc):
            stats = smpool.tile([128, 6], F32, tag="lnstats")
            nc.vector.bn_stats(out=stats, in_=x_sl)
            mv = smpool.tile([128, 2], F32, tag="lnmv")
            nc.vector.bn_aggr(out=mv, in_=stats)
            rstd = smpool.tile([128, 1], F32, tag="lnrstd")
            nc.scalar.activation(out=rstd, in_=mv[:, 1:2], func=ACTF.Sqrt,
                                 bias=eps, scale=1.0)
            nc.vector.reciprocal(out=rstd, in_=rstd)
            nc.vector.tensor_scalar(out=x_sl, in0=x_sl, scalar1=mv[:, 0:1],
                                    scalar2=rstd, op0=ALU.subtract, op1=ALU.mult)
            nc.vector.tensor_tensor(out=x_sl, in0=x_sl, in1=g_bc, op=ALU.mult)
            nc.vector.tensor_tensor(out=x_sl, in0=x_sl, in1=b_bc, op=ALU.add)

        def transpose_to_bf16(src_f32, dst_bf):
            # src [128, TC, 512] f32 -> dst [128, DC, TOK] bf16 (transposed)
            for tcc in range(TC):
                for dc in range(DC):
                    pt = ppool.tile([128, 128], F32, tag="trf")
                    nc.tensor.transpose(pt, src_f32[:, tcc, dc * 128:(dc + 1) * 128], idf)
                    nc.vector.tensor_copy(out=dst_bf[:, dc, tcc * 128:(tcc + 1) * 128], in_=pt)

        # ---------------- front-end (replicated) ----------------
        alphas = cpool.tile([128, B], F32)
        X = xpool.tile([128, TC, D], F32, tag="X")
        fe_ctx = contextlib.ExitStack()
        with fe_ctx:
            fpool = fe_ctx.enter_context(tc_.tile_pool(name="fpool", bufs=1))
            f2pool = fe_ctx.enter_context(tc_.tile_pool(name="f2pool", bufs=2))

            few = {}
            for n in ["lfc1_w", "lfc2_w", "gq_w", "gk_w", "gfc_w", "ffc_w", "ofc_w"]:
                t = fpool.tile(list(prm[n].shape), F32, tag=n)
                nc.sync.dma_start(out=t, in_=prm[n][:, :])
                few[n] = t
            feb = {}
            for n in ["lfc1_b", "lfc2_b", "gq_b", "gk_b", "gfc_b", "ffc_b"]:
                t = fpool.tile([HID, 1], F32, tag=n)
                nc.sync.dma_start(out=t, in_=prm[n].ap()[:, None])
                feb[n] = t
            ofcb = fpool.tile([128, 1], F32)
            src = prm["ofc_b"].ap()
            nc.sync.dma_start(out=ofcb, in_=bass.AP(tensor=src.tensor, offset=src.offset,
                                                    ap=[[0, 128]] + src.ap))
            gfcb_bc = bc_tile(fpool, "gfc_b", HID, tag="gfcb_bc")
            hng_bc = bc_tile(fpool, "hn_g", HID)
            hnb_bc = bc_tile(fpool, "hn_b", HID)
            neT = fpool.tile([HID, C], F32)
            nc.sync.dma_start(out=neT, in_=prm["node_embT"][:, :])
            eye55 = fpool.tile([C, C], F32)
            nc.sync.dma_start(out=eye55, in_=prm["eye55"][:, :])
            tokw_a = fpool.tile([128, D], F32)
            nc.sync.dma_start(out=tokw_a, in_=prm["tokw"][0:128, :])
            tokw_b = fpool.tile([64, D], F32)
            nc.sync.dma_start(out=tokw_b, in_=prm["tokw"][128:192, :])

            # simp = 0.2*relu(ne@neT) + eye
            pt = ppool.tile([128, 512], F32, tag="mm")
            nc.tensor.matmul(pt[:C, :C], neT, neT, start=True, stop=True)
            simp = fpool.tile([C, C], F32)
            nc.scalar.activation(out=simp, in_=pt[:C, :C], func=ACTF.Relu)
            nc.vector.tensor_scalar_mul(out=simp, in0=simp, scalar1=0.2)
            nc.vector.tensor_tensor(out=simp, in0=simp, in1=eye55, op=ALU.add)

            def fe_mm(lhsT, rhs, m, n, func=None, bias=None, tag="fe"):
                ptm = ppool.tile([128, 512], F32, tag="mm")
                nc.tensor.matmul(ptm[:m, :n], lhsT, rhs, start=True, stop=True)
                ot = f2pool.tile([m, n], F32, tag=f"fe_{tag}")
                if func is None:
                    nc.vector.tensor_copy(out=ot, in_=ptm[:m, :n])
                else:
                    nc.scalar.activation(out=ot, in_=ptm[:m, :n], func=func,
                                         bias=bias if bias is not None else 0.0)
                return ot

            def fe_tr(in_, m, n, poff=0, tag="tr"):
                # in_ [m, n] at base partition poff -> out tile [n, m] at base 0
                ptm = ppool.tile([128, 512], F32, tag="mm")
                nc.tensor.transpose(ptm[:n, :m], in_, idf[poff:poff + m, poff:poff + m])
                ot = f2pool.tile([n, m], F32, tag=f"fetr_{tag}")
                nc.vector.tensor_copy(out=ot, in_=ptm[:n, :m])
                return ot

            for bp in range(2):
                xp = f2pool.tile([128, L], F32, tag="xp")
                nc.vector.memset(xp, 0.0)
                for half in range(2):
                    b = 2 * bp + half
                    nc.sync.dma_start(out=xp[64 * half:64 * half + C, :], in_=prm["xT"][b])
                acc = f2pool.tile([128, L], F32, tag="acc")
                nc.vector.tensor_tensor(out=acc[:, 1:], in0=xp[:, 1:], in1=xp[:, :L - 1], op=ALU.add)
                nc.vector.tensor_tensor(out=acc[:, 0:1], in0=xp[:, 0:1], in1=xp[:, 0:1], op=ALU.add)
                nc.vector.tensor_tensor(out=acc[:, :L - 1], in0=acc[:, :L - 1], in1=xp[:, 1:], op=ALU.add)
                nc.vector.tensor_tensor(out=acc[:, L - 1:], in0=acc[:, L - 1:], in1=xp[:, L - 1:], op=ALU.add)
                nc.vector.tensor_tensor(out=acc[:, 2:], in0=acc[:, 2:], in1=xp[:, :L - 2], op=ALU.add)
                nc.vector.tensor_tensor(out=acc[:, 0:2], in0=acc[:, 0:2],
                                        in1=xp[:, 0:1].to_broadcast([128, 2]), op=ALU.add)
                nc.vector.tensor_tensor(out=acc[:, :L - 2], in0=acc[:, :L - 2], in1=xp[:, 2:], op=ALU.add)
                nc.vector.tensor_tensor(out=acc[:, L - 2:], in0=acc[:, L - 2:],
                                        in1=xp[:, L - 1:].to_broadcast([128, 2]), op=ALU.add)
                res = f2pool.tile([128, L], F32, tag="res")
                nc.vector.tensor_scalar_mul(out=res, in0=acc, scalar1=0.2)
                nc.vector.tensor_tensor(out=res, in0=xp, in1=res, op=ALU.subtract)
                a = f2pool.tile([128, L], F32, tag="absres")
                nc.scalar.activation(out=a, in_=res, func=ACTF.Abs)
                stats = smpool.tile([128, 6], F32, tag="festats")
                nc.vector.bn_stats(out=stats, in_=a)
                mv = smpool.tile([128, 2], F32, tag="femv")
                nc.vector.bn_aggr(out=mv, in_=stats)
                mx = smpool.tile([128, 1], F32, tag="femx")
                nc.vector.tensor_reduce(out=mx, in_=a, axis=AX.X, op=ALU.max)
                h0 = f2pool.tile([128, 4], F32, tag="h0")
                nc.gpsimd.tensor_copy(out=h0[:, 0:1], in_=mv[:, 0:1])
                nc.gpsimd.tensor_copy(out=h0[:, 1:2], in_=mx)
                nc.scalar.activation(out=h0[:, 2:3], in_=mv[:, 1:2], func=ACTF.Sqrt)
                m1sq = smpool.tile([128, 1], F32, tag="fem1")
                nc.scalar.activation(out=m1sq, in_=mv[:, 0:1], func=ACTF.Square)
                nc.vector.tensor_tensor(out=h0[:, 3:4], in0=mv[:, 1:2], in1=m1sq, op=ALU.add)

                for half in range(2):
                    b = 2 * bp + half
                    off = 64 * half
                    h0T = fe_tr(h0[off:off + C, :], C, 4, poff=off, tag="h0T")  # [4, 55]
                    z1T = fe_mm(few["lfc1_w"], h0T, HID, C, ACTF.Relu, feb["lfc1_b"], tag="z1T")
                    hlocT = fe_mm(few["lfc2_w"], z1T, HID, C, ACTF.Identity, feb["lfc2_b"], tag="hlocT")
                    qT = fe_mm(few["gq_w"], h0T, HID, C, ACTF.Identity, feb["gq_b"], tag="qT")
                    kT = fe_mm(few["gk_w"], h0T, HID, C, ACTF.Identity, feb["gk_b"], tag="kT")
                    hgT = fe_mm(few["gfc_w"], h0T, HID, C, ACTF.Identity, feb["gfc_b"], tag="hgT")
                    hg = fe_mm(h0T, few["gfc_w"], C, HID, tag="hg")
                    nc.vector.tensor_tensor(out=hg, in0=hg, in1=gfcb_bc[:C, :HID], op=ALU.add)
                    Ain = fe_mm(qT, kT, C, C, ACTF.Relu, tag="Ain")
                    nc.vector.tensor_tensor(out=Ain, in0=Ain, in1=simp, op=ALU.add)
                    rsum = smpool.tile([C, 1], F32, tag="fersum")
                    A = f2pool.tile([C, C], F32, tag="A")
                    nc.scalar.activation(out=A, in_=Ain, func=ACTF.Exp, accum_out=rsum)
                    nc.vector.reciprocal(out=rsum, in_=rsum)
                    nc.vector.tensor_scalar_mul(out=A, in0=A, scalar1=rsum)
                    AT = fe_tr(A, C, C, tag="AT")
                    AhgT = fe_mm(hg, AT, HID, C, ACTF.Relu, tag="AhgT")
                    nc.vector.tensor_scalar_mul(out=AhgT, in0=AhgT, scalar1=0.3)
                    nc.vector.tensor_tensor(out=AhgT, in0=AhgT, in1=hgT, op=ALU.add)
                    nc.vector.tensor_scalar_mul(out=AhgT, in0=AhgT, scalar1=0.1)
                    nc.vector.tensor_tensor(out=AhgT, in0=AhgT, in1=hlocT, op=ALU.add)
                    fuse = fe_tr(AhgT, HID, C, tag="fuse")                      # [55, 16]
                    st2 = smpool.tile([C, 6], F32, tag="fest2")
                    nc.vector.bn_stats(out=st2, in_=fuse)
                    mv2 = smpool.tile([C, 2], F32, tag="femv2")
                    nc.vector.bn_aggr(out=mv2, in_=st2)
                    rstd = smpool.tile([C, 1], F32, tag="ferstd")
                    nc.scalar.activation(out=rstd, in_=mv2[:, 1:2], func=ACTF.Sqrt,
                                         bias=eps[:C], scale=1.0)
                    nc.vector.reciprocal(out=rstd, in_=rstd)
                    nc.vector.tensor_scalar(out=fuse, in0=fuse, scalar1=mv2[:, 0:1],
                                            scalar2=rstd, op0=ALU.subtract, op1=ALU.mult)
                    nc.vector.tensor_tensor(out=fuse, in0=fuse, in1=hng_bc[:C, :], op=ALU.mult)
                    nc.vector.tensor_tensor(out=fuse, in0=fuse, in1=hnb_bc[:C, :], op=ALU.add)
                    hnT = fe_tr(fuse, C, HID, tag="hnT")                       # [16, 55]
                    hT = fe_mm(few["ffc_w"], hnT, HID, C, ACTF.Relu, feb["ffc_b"], tag="hT")
                    pal = ppool.tile([128, 512], F32, tag="mm")
                    nc.tensor.matmul(pal[:C, :1], hT, few["ofc_w"], start=True, stop=True)
                    alf = smpool.tile([C, 1], F32, tag="fealf")
                    nc.scalar.activation(out=alf, in_=pal[:C, :1], func=ACTF.Tanh,
                                         bias=ofcb[:C], scale=1.0)
                    nc.vector.tensor_scalar(out=alphas[:C, b:b + 1], in0=alf, scalar1=0.3,
                                            scalar2=1.2, op0=ALU.mult, op1=ALU.add)

            # windowed enhancement + circular-conv token embedding -> X
            for b in range(B):
                xw = f2pool.tile([C, 5, LS + 2], F32, tag="xw")
                nc.sync.dma_start(out=xw, in_=prm["xw5T"][b].rearrange("t c w -> c t w"))
                bgw = f2pool.tile([C, LS + 2], F32, tag="bgw")
                nc.vector.tensor_tensor(out=bgw, in0=xw[:, 0, :], in1=xw[:, 1, :], op=ALU.add)
                nc.vector.tensor_tensor(out=bgw, in0=bgw, in1=xw[:, 2, :], op=ALU.add)
                nc.vector.tensor_tensor(out=bgw, in0=bgw, in1=xw[:, 3, :], op=ALU.add)
                nc.vector.tensor_tensor(out=bgw, in0=bgw, in1=xw[:, 4, :], op=ALU.add)
                nc.vector.tensor_scalar_mul(out=bgw, in0=bgw, scalar1=0.2)
                nc.vector.tensor_tensor(out=bgw, in0=xw[:, 2, :], in1=bgw, op=ALU.subtract)
                nc.vector.tensor_scalar_mul(out=bgw, in0=bgw, scalar1=alphas[:C, b:b + 1])
                enh = f2pool.tile([C, LS + 2], F32, tag="enh")
                nc.vector.tensor_tensor(out=enh, in0=xw[:, 2, :], in1=bgw, op=ALU.add)
                x3a = f2pool.tile([128, LS], F32, tag="x3a")
                nc.vector.memset(x3a, 0.0)
                x3b = f2pool.tile([64, LS], F32, tag="x3b")
                nc.vector.memset(x3b, 0.0)
                nc.gpsimd.tensor_copy(out=x3a[0:C, :], in_=enh[:, 0:LS])
                nc.gpsimd.tensor_copy(out=x3a[64:64 + C, :], in_=enh[:, 1:LS + 1])
                nc.gpsimd.tensor_copy(out=x3b[0:C, :], in_=enh[:, 2:LS + 2])
                pt0 = ppool.tile([128, 512], F32, tag="mm")
                nc.tensor.matmul(pt0[:LS, :], x3a, tokw_a, start=True, stop=False)
                nc.tensor.matmul(pt0[:LS, :], x3b, tokw_b, start=False, stop=True)
                off = 64 * (b % 2)
                nc.vector.tensor_tensor(out=X[off:off + LS, b // 2, :], in0=pt0[:LS, :],
                                        in1=pedup[off:off + LS, :], op=ALU.add)

        # debug taps: X post-embedding, alphas
        nc.sync.dma_start(out=outs["o_dbg"][0], in_=X[:, 0, :])
        nc.sync.dma_start(out=outs["o_dbg"][1], in_=X[:, 1, :])
        nc.sync.dma_start(out=outs["o_dbg"][2][:, 0:B], in_=alphas)

        # ---------------- transformer layers ----------------
        for li in range(NL):
            w = {}
            for n in ["wq", "wk", "wv", "wo", "c1", "c2"]:
                t = wpool.tile([128, DC, D], BF16, tag=n)
                nc.sync.dma_start(out=t, in_=prm[f"{n}{li}"].ap().rearrange("(dc p) n -> p dc n", p=128))
                w[n] = t
            wsig = wpool.tile([128, DC, H], BF16, tag="wsig")
            nc.sync.dma_start(out=wsig, in_=prm[f"wsig{li}"].ap().rearrange("(dc p) n -> p dc n", p=128))
            brow = {}
            for n in ["bq", "bk", "bv", "bo", "c1b", "c2b", "bsig"]:
                t = lnpool.tile([1, D if n != "bsig" else H], BF16, tag=f"br_{n}")
                nc.sync.dma_start(out=t, in_=prm[f"{n}{li}"][:, :])
                brow[n] = t
            lnbc = {}
            for n in ["ln1g", "ln1b", "ln2g", "ln2b"]:
                lnbc[n] = bc_tile(lnpool, f"{n}{li}", D, F32, tag=f"bc_{n}")

            XT = s2pool.tile([128, DC, TOK], BF16, tag="XT")
            transpose_to_bf16(X, XT)

            # projections on own tokens
            QT = s2pool.tile([128, DC, TOK], BF16, tag="QT")
            KTo = s2pool.tile([128, DC, TOK], BF16, tag="KTo")
            for name, dst, bn in [("wq", QT, "bq"), ("wk", KTo, "bk")]:
                for dco in range(DC):
                    pt1 = ppool.tile([128, 512], F32, tag="mm")
                    for dci in range(DC):
                        nc.tensor.matmul(pt1[:, :TOK], w[name][:, dci, dco * 128:(dco + 1) * 128],
                                         XT[:, dci, :], start=(dci == 0), stop=False)
                    nc.tensor.matmul(pt1[:, :TOK], brow[bn][:, dco * 128:(dco + 1) * 128],
                                     ones_bf[:, :TOK], start=False, stop=True)
                    nc.vector.tensor_copy(out=dst[:, dco, :], in_=pt1[:, :TOK])
            Vo = s2pool.tile([128, TC, D], BF16, tag="Vo")
            for tcc in range(TC):
                pt2 = ppool.tile([128, 512], F32, tag="mm")
                for dci in range(DC):
                    nc.tensor.matmul(pt2, XT[:, dci, tcc * 128:(tcc + 1) * 128], w["wv"][:, dci, :],
                                     start=(dci == 0), stop=False)
                nc.tensor.matmul(pt2, ones_bf[:, tcc * 128:(tcc + 1) * 128], brow["bv"],
                                 start=False, stop=True)
                nc.vector.tensor_copy(out=Vo[:, tcc, :], in_=pt2)

            # ship own K/V and all-gather (rank r rows [1024r, 1024r+1024) of agout)
            nc.sync.dma_start(out=agin[0:512, :].rearrange("(dc p) t -> p dc t", p=128), in_=KTo)
            nc.sync.dma_start(out=agin[512:1024, :].rearrange("(p t j) c -> p t (j c)", p=128, t=2),
                              in_=Vo)
            nc.gpsimd.collective_compute(
                "AllGather", ALU.bypass,
                replica_groups=[list(range(NCORES))],
                ins=[agin[:, :]], outs=[agout[:, :]],
            )

            # sigma / prior / sigma-out (independent of the collective)
            sig_t = s2pool.tile([128, TC, H], F32, tag="sig")
            for tcc in range(TC):
                pt3 = ppool.tile([128, 512], F32, tag="mm")
                for dci in range(DC):
                    nc.tensor.matmul(pt3[:, :H], XT[:, dci, tcc * 128:(tcc + 1) * 128],
                                     wsig[:, dci, :], start=(dci == 0), stop=False)
                nc.tensor.matmul(pt3[:, :H], ones_bf[:, tcc * 128:(tcc + 1) * 128], brow["bsig"],
                                 start=False, stop=True)
                nc.scalar.activation(out=sig_t[:, tcc, :], in_=pt3[:, :H], func=ACTF.Sigmoid, scale=5.0)
                nc.scalar.activation(out=sig_t[:, tcc, :], in_=sig_t[:, tcc, :], func=ACTF.Exp,
                                     scale=LN3, bias=b3eps)
                nc.vector.tensor_scalar(out=sig_t[:, tcc, :], in0=sig_t[:, tcc, :], scalar1=1.0,
                                        scalar2=1.0, op0=ALU.subtract, op1=ALU.mult)
            if li == 0:
                nc.sync.dma_start(out=outs["o_dbg"][3][:, 0:TC * H],
                                  in_=sig_t.rearrange("p a b -> p (a b)"))
            s1 = s2pool.tile([128, TC * H], F32, tag="s1")
            ccn = s2pool.tile([128, TC * H], F32, tag="ccn")
            sflat = sig_t.rearrange("p a b -> p (a b)")
            nc.scalar.activation(out=s1, in_=sflat, func=ACTF.Square)
            nc.vector.reciprocal(out=s1, in_=s1)
            nc.vector.tensor_scalar_mul(out=s1, in0=s1, scalar1=-0.5)
            nc.scalar.activation(out=ccn, in_=sflat, func=ACTF.Ln, scale=math.sqrt(2.0 * math.pi))
            nc.vector.tensor_scalar_mul(out=ccn, in0=ccn, scalar1=-1.0)
            for h in range(H):
                for tcc in range(TC):
                    col = tcc * H + h
                    pr = qpool.tile([128, L], F32, tag="prsg")
                    nc.scalar.activation(out=pr, in_=d2dup, func=ACTF.Exp,
                                         scale=s1[:, col:col + 1], bias=ccn[:, col:col + 1])
                    nc.sync.dma_start(out=outs["o_prior"][li, h, tcc], in_=pr)
                    sg = qpool.tile([128, L], F32, tag="prsg")
                    nc.vector.tensor_scalar(out=sg, in0=d2dup, scalar1=0.0,
                                            scalar2=sig_t[:, tcc, h:h + 1],
                                            op0=ALU.mult, op1=ALU.add)
                    nc.sync.dma_start(out=outs["o_sigma"][li, h, tcc], in_=sg)

            # gathered K/V into SBUF
            KT = kvpool.tile([128, DC, B * L], BF16, tag="KT")      # cols r*256+b*64+l'
            for r in range(NCORES):
                nc.sync.dma_start(
                    out=KT[:, :, r * 256:(r + 1) * 256],
                    in_=agout[1024 * r:1024 * r + 512, :].rearrange("(dc p) t -> p dc t", p=128))
            V = kvpool.tile([128, B * 4, D], BF16, tag="V")         # [s-part, (b,sc), d]
            for r in range(NCORES):
                src = agout[1024 * r + 512:1024 * (r + 1), :].rearrange(
                    "(h2 l t j) c -> l t h2 (j c)", h2=2, l=64, t=2, j=2)
                dst = V[64 * (r % 2):64 * (r % 2) + 64].rearrange(
                    "p (t h2 rr) d -> p t h2 rr d", t=2, h2=2)[:, :, :, r // 2, :]
                nc.sync.dma_start(out=dst, in_=src)

            # attention per (h, tc)
            newxT = s2pool.tile([128, DC, TOK], BF16, tag="newxT")
            for h in range(H):
                hp = 64 * (h % 2)
                hc = h // 2
                for tcc in range(TC):
                    sc_ps = ppool.tile([128, 512], F32, tag="mm")
                    for half in range(2):
                        b = 2 * tcc + half
                        rhs = KT[hp:hp + 64, hc].rearrange("p (r q) -> p r q", q=256)[:, :, b * 64:b * 64 + 64]
                        nc.tensor.matmul(sc_ps[64 * half:64 * half + 64, :],
                                         QT[hp:hp + 64, hc,
                                            tcc * 128 + 64 * half:tcc * 128 + 64 * half + 64],
                                         rhs, start=True, stop=True)
                    Praw = qpool.tile([128, L], F32, tag="Praw")
                    rsum = smpool.tile([128, 1], F32, tag="rsum")
                    nc.scalar.activation(out=Praw, in_=sc_ps, func=ACTF.Exp, scale=SCALE,
                                         accum_out=rsum)
                    nc.vector.reciprocal(out=rsum, in_=rsum)
                    ser = qpool.tile([128, L], F32, tag="ser")
                    nc.vector.tensor_scalar_mul(out=ser, in0=Praw, scalar1=rsum)
                    nc.sync.dma_start(out=outs["o_series"][li, h, tcc], in_=ser)
                    Pn = qpool.tile([128, L], BF16, tag="Pn")
                    nc.vector.tensor_scalar_mul(out=Pn, in0=Praw, scalar1=rsum)
                    PT = qpool.tile([128, 4, 128], BF16, tag="PT")
                    ptb = ppool.tile([128, 4, 128], BF16, tag="trb")
                    for sc in range(4):
                        nc.tensor.transpose(ptb[:, sc, :], Pn[:, sc * 128:(sc + 1) * 128], idb)
                    nc.scalar.copy(out=PT, in_=ptb)
                    for half in range(2):
                        b = 2 * tcc + half
                        av = ppool.tile([64, 64], F32, tag="av")
                        for sc in range(4):
                            nc.tensor.matmul(av, V[:, 4 * b + sc, 64 * h:64 * h + 64],
                                             PT[:, sc, 64 * half:64 * half + 64],
                                             start=(sc == 0), stop=(sc == 3))
                        nc.vector.tensor_copy(
                            out=newxT[hp:hp + 64, hc,
                                      tcc * 128 + 64 * half:tcc * 128 + 64 * half + 64],
                            in_=av)

            # wo + residual + LN1
            xr = s2pool.tile([128, TC, D], F32, tag="xr")
            for tcc in range(TC):
                pt4 = ppool.tile([128, 512], F32, tag="mm")
                for dci in range(DC):
                    nc.tensor.matmul(pt4, newxT[:, dci, tcc * 128:(tcc + 1) * 128], w["wo"][:, dci, :],
                                     start=(dci == 0), stop=False)
                nc.tensor.matmul(pt4, ones_bf[:, tcc * 128:(tcc + 1) * 128], brow["bo"],
                                 start=False, stop=True)
                nc.vector.tensor_tensor(out=xr[:, tcc, :], in0=pt4, in1=X[:, tcc, :], op=ALU.add)
                ln_inplace(xr[:, tcc, :], lnbc["ln1g"], lnbc["ln1b"])

            # FFN + LN2 (stage 1 in transposed dff-major layout)
            xrT = s2pool.tile([128, DC, TOK], BF16, tag="xrT")
            transpose_to_bf16(xr, xrT)
            y1T = s2pool.tile([128, DC, TOK], BF16, tag="y1T")
            for dfo in range(DC):
                pt5 = ppool.tile([128, 512], F32, tag="mm")
                for dci in range(DC):
                    nc.tensor.matmul(pt5[:, :TOK], w["c1"][:, dci, dfo * 128:(dfo + 1) * 128],
                                     xrT[:, dci, :], start=(dci == 0), stop=False)
                nc.tensor.matmul(pt5[:, :TOK], brow["c1b"][:, dfo * 128:(dfo + 1) * 128],
                                 ones_bf[:, :TOK], start=False, stop=True)
                tg = qpool.tile([128, TOK], F32, tag="tg")
                nc.scalar.activation(out=tg, in_=pt5[:, :TOK], func=ACTF.Erf,
                                     scale=1.0 / math.sqrt(2.0))
                nc.vector.tensor_scalar(out=tg, in0=tg, scalar1=1.0, scalar2=0.5,
                                        op0=ALU.add, op1=ALU.mult)
                nc.vector.tensor_tensor(out=y1T[:, dfo, :], in0=tg, in1=pt5[:, :TOK],
                                        op=ALU.mult)
            Xn = xpool.tile([128, TC, D], F32, tag="X")
            for tcc in range(TC):
                pt6 = ppool.tile([128, 512], F32, tag="mm")
                for dci in range(DC):
                    nc.tensor.matmul(pt6, y1T[:, dci, tcc * 128:(tcc + 1) * 128], w["c2"][:, dci, :],
                                     start=(dci == 0), stop=False)
                nc.tensor.matmul(pt6, ones_bf[:, tcc * 128:(tcc + 1) * 128], brow["c2b"],
                                 start=False, stop=True)
                nc.vector.tensor_tensor(out=Xn[:, tcc, :], in0=pt6, in1=xr[:, tcc, :], op=ALU.add)
                ln_inplace(Xn[:, tcc, :], lnbc["ln2g"], lnbc["ln2b"])
            X = Xn

        # ---------------- final projection ----------------
        lnfg_bc = bc_tile(cpool, "lnfg", D)
        lnfb_bc = bc_tile(cpool, "lnfb", D)
        projw = cpool.tile([128, DC, C], BF16)
        nc.sync.dma_start(out=projw, in_=prm["projw"].ap().rearrange("(dc p) n -> p dc n", p=128))
        projb = cpool.tile([1, C], BF16)
        nc.sync.dma_start(out=projb, in_=prm["projb"][:, :])
        for tcc in range(TC):
            ln_inplace(X[:, tcc, :], lnfg_bc, lnfb_bc)
        XfT = s2pool.tile([128, DC, TOK], BF16, tag="XfT")
        transpose_to_bf16(X, XfT)
        for tcc in range(TC):
            pt7 = ppool.tile([128, 512], F32, tag="mm")
            for dci in range(DC):
                nc.tensor.matmul(pt7[:, :C], XfT[:, dci, tcc * 128:(tcc + 1) * 128],
                                 projw[:, dci, :], start=(dci == 0), stop=False)
            nc.tensor.matmul(pt7[:, :C], ones_bf[:, tcc * 128:(tcc + 1) * 128], projb,
                             start=False, stop=True)
            fo = qpool.tile([128, C], F32, tag="fo")
            nc.vector.tensor_copy(out=fo, in_=pt7[:, :C])
            nc.sync.dma_start(out=outs["o_final"][tcc], in_=fo)


# ======================= host side =======================

_NC = None


def _get_nc():
    global _NC
    if _NC is None:
        _NC = build()
    return _NC


def _host_inputs(x, params):
    """Build the replicated + per-core input maps."""
    p = params
    rep = {}
    rep["xT"] = np.ascontiguousarray(np.asarray(x, np.float32).transpose(0, 2, 1))

    rep["node_embT"] = np.ascontiguousarray(np.asarray(p["node_emb"], np.float32).T)
    rep["eye55"] = np.eye(C, dtype=np.float32)
    for n in ["lfc1_w", "lfc2_w", "gq_w", "gk_w", "gfc_w", "ffc_w", "ofc_w",
              "lfc1_b", "lfc2_b", "gq_b", "gk_b", "gfc_b", "ffc_b", "hn_g", "hn_b", "ofc_b"]:
        rep[n] = np.ascontiguousarray(np.asarray(p[n], np.float32))

    tokw = np.zeros((192, D), np.float32)
    tw = np.asarray(p["tok_w"], np.float32)            # [3, C, D] (WIO)
    for wtap in range(3):
        tokw[64 * wtap:64 * wtap + C] = tw[wtap]
    rep["tokw"] = tokw

    for i, lp in enumerate(p["layers"]):
        for n, k in [("wq", "wq"), ("wk", "wk"), ("wv", "wv"), ("wo", "wo"),
                     ("c1", "c1_w"), ("c2", "c2_w")]:
            rep[f"{n}{i}"] = np.asarray(lp[k], np.float32).astype(BF)
        rep[f"wsig{i}"] = np.asarray(lp["wsig"], np.float32).astype(BF)
        for n, k in [("bq", "bq"), ("bk", "bk"), ("bv", "bv"), ("bo", "bo"),
                     ("c1b", "c1_b"), ("c2b", "c2_b")]:
            rep[f"{n}{i}"] = np.asarray(lp[k], np.float32).astype(BF).reshape(1, -1)
        rep[f"bsig{i}"] = np.asarray(lp["bsig"], np.float32).astype(BF).reshape(1, -1)
        for n, k in [("ln1g", "ln1_g"), ("ln1b", "ln1_b"), ("ln2g", "ln2_g"), ("ln2b", "ln2_b")]:
            rep[f"{n}{i}"] = np.asarray(lp[k], np.float32)
    rep["lnfg"] = np.asarray(p["lnf_g"], np.float32)
    rep["lnfb"] = np.asarray(p["lnf_b"], np.float32)
    rep["projw"] = np.asarray(p["proj_w"], np.float32).astype(BF)
    rep["projb"] = np.asarray(p["proj_b"], np.float32).astype(BF).reshape(1, -1)

    # positional embedding rows (constant, replicated formula)
    pos = np.arange(L, dtype=np.float32)[:, None]
    div = np.exp(np.arange(0, D, 2, dtype=np.float32) * (-math.log(10000.0) / D))
    pe = np.zeros((L, D), np.float32)
    pe[:, 0::2] = np.sin(pos * div)
    pe[:, 1::2] = np.cos(pos * div)
    dist = np.abs(np.arange(L, dtype=np.float32)[:, None] - np.arange(L, dtype=np.float32)[None, :])
    d2 = dist.astype(np.float32) ** 2

    xf = np.asarray(x, np.float32)
    in_maps = []
    for i in range(NCORES):
        m = dict(rep)
        rows = np.arange(LS * i, LS * i + LS)
        m["pedup"] = np.ascontiguousarray(
            np.concatenate([pe[rows], pe[rows]], axis=0))
        m["d2dup"] = np.ascontiguousarray(
            np.concatenate([d2[rows], d2[rows]], axis=0))
        # window: positions j=0..65 are l = 64i-1+j (mod L); 5 clamped bg taps
        win = (np.arange(LS * i - 1, LS * i + LS + 1)) % L      # [66]
        xw5 = np.empty((B, 5, C, LS + 2), np.float32)
        for t in range(5):
            idx = np.clip(win + (t - 2), 0, L - 1)
            xw5[:, t] = xf[:, idx, :].transpose(0, 2, 1)
        m["xw5T"] = np.ascontiguousarray(xw5)
        in_maps.append(m)
    return in_maps


def kernel(x, params):
    from concourse.bass_utils import run_bass_kernel_spmd

    nc = _get_nc()
    in_maps = _host_inputs(x, params)
    res = run_bass_kernel_spmd(nc, in_maps, list(range(NCORES)))
    results = res.results

    series = np.empty((NL, B, H, L, L), np.float32)
    prior = np.empty((NL, B, H, L, L), np.float32)
    sigma = np.empty((NL, B, H, L, L), np.float32)
    out = np.empty((B, L, C), np.float32)
    for i in range(NCORES):
        r = results[i]
        # o_* [NL, H, TC, 128, L]; partition p = 64*(b%2)+l', tc = b//2
        for name, dstf in [("o_series", series), ("o_prior", prior), ("o_sigma", sigma)]:
            v = r[name]
            for b in range(B):
                sl = v[:, :, b // 2, 64 * (b % 2):64 * (b % 2) + 64, :]     # [NL,H,64,L]
                dstf[:, b, :, LS * i:LS * i + LS, :] = sl
        vf = r["o_final"]
        for b in range(B):
            out[b, LS * i:LS * i + LS, :] = vf[b // 2, 64 * (b % 2):64 * (b % 2) + 64, :]
    return (out, series, prior, sigma)


# revision 20
# speedup vs baseline: 1.0753x; 1.0753x over previous
"""AnomalyTransformer Trainium2 kernel: 8-core SPMD, sequence-parallel over L.

Core i owns L-rows [64i, 64i+64) for all batches. Per layer each core computes
K/V for its own token rows and the cores AllGather them (bf16, 512KB/rank);
scores, softmax(series), Gaussian prior, sigma, attention output, FFN and
LayerNorms run on owned rows only. The tiny front-end enhancement is
replicated on every core; per-core "window" inputs carry the circularly /
edge-padded x slices so all cores run one identical program (SPMD).
"""

import contextlib
import math
import sys

sys.path.insert(0, "/opt/trn_rl_repo")

import numpy as np
import ml_dtypes

import concourse.bass as bass
import concourse.tile as tile
from concourse import mybir, bacc
from concourse.masks import make_identity

F32 = mybir.dt.float32
BF16 = mybir.dt.bfloat16
AX = mybir.AxisListType
ALU = mybir.AluOpType
ACTF = mybir.ActivationFunctionType

NCORES = 8
B, L, C = 4, 512, 55
D, H, DH, DFF, NL = 512, 8, 64, 512, 3
LS = L // NCORES          # 64 own rows per core
TOK = B * LS              # 256 own tokens
TC = TOK // 128           # 2 token chunks
DC = D // 128             # 4 D chunks
HID = 16
SCALE = 1.0 / math.sqrt(DH)
LN3 = math.log(3.0)
INV_SQRT2PI = 1.0 / math.sqrt(2.0 * math.pi)

BF = ml_dtypes.bfloat16


def _declare_params(nc):
    p = {}

    def di(name, shape, dt=F32):
        p[name] = nc.declare_dram_parameter(name, list(shape), dt, isOutput=False)

    # per-core varying
    di("xw5T", (B, 5, C, LS + 2))
    di("d2dup", (128, L))
    di("pedup", (128, D))
    # replicated
    di("xT", (B, C, L))
    di("node_embT", (HID, C))
    di("eye55", (C, C))
    for n, shape in [
        ("lfc1_w", (4, HID)), ("lfc2_w", (HID, HID)), ("gq_w", (4, HID)),
        ("gk_w", (4, HID)), ("gfc_w", (4, HID)), ("ffc_w", (HID, HID)),
        ("ofc_w", (HID, 1)),
    ]:
        di(n, shape)
    for n in ["lfc1_b", "lfc2_b", "gq_b", "gk_b", "gfc_b", "ffc_b",
              "hn_g", "hn_b"]:
        di(n, (HID,))
    di("ofc_b", (1,))
    di("tokw", (192, D))          # f32, taps at 64-row pitch, zero padded
    for i in range(NL):
        for n in ["wq", "wk", "wv", "wo", "c1", "c2"]:
            di(f"{n}{i}", (D, D), BF16)
        di(f"wsig{i}", (D, H), BF16)
        for n in ["bq", "bk", "bv", "bo", "c1b", "c2b"]:
            di(f"{n}{i}", (1, D), BF16)
        di(f"bsig{i}", (1, H), BF16)
        for n in ["ln1g", "ln1b", "ln2g", "ln2b"]:
            di(f"{n}{i}", (D,))
    di("lnfg", (D,))
    di("lnfb", (D,))
    di("projw", (D, C), BF16)
    di("projb", (1, C), BF16)

    outs = {
        "o_dbg": nc.declare_dram_parameter("o_dbg", [8, 128, 512], F32, isOutput=True),
        "o_series": nc.declare_dram_parameter("o_series", [NL, H, TC, 128, L], F32, isOutput=True),
        "o_prior": nc.declare_dram_parameter("o_prior", [NL, H, TC, 128, L], F32, isOutput=True),
        "o_sigma": nc.declare_dram_parameter("o_sigma", [NL, H, TC, 128, L], F32, isOutput=True),
        "o_final": nc.declare_dram_parameter("o_final", [TC, 128, C], F32, isOutput=True),
    }
    return p, outs


def build():
    nc = bacc.Bacc()
    prm, outs = _declare_params(nc)
    agin = nc.dram_tensor("agin", [1024, 256], BF16)
    agout = nc.dram_tensor("agout", [8192, 256], BF16, addr_space="Shared")
    wuin = nc.dram_tensor("wuin", [1, 64], F32)
    wuout = nc.dram_tensor("wuout", [8, 64], F32, addr_space="Shared")
    with tile.TileContext(nc) as tc_:
        _body(nc, tc_, prm, outs, agin, agout, wuin, wuout)
    nc.finalize()
    return nc


def _body(nc, tc_, prm, outs, agin, agout, wuin, wuout):
    ctx = contextlib.ExitStack()
    with ctx:
        cpool = ctx.enter_context(tc_.tile_pool(name="cpool", bufs=1))
        wpool = ctx.enter_context(tc_.tile_pool(name="wpool", bufs=2))
        lnpool = ctx.enter_context(tc_.tile_pool(name="lnpool", bufs=1))
        xpool = ctx.enter_context(tc_.tile_pool(name="xpool", bufs=2))
        s2pool = ctx.enter_context(tc_.tile_pool(name="s2pool", bufs=1))
        kvpool = ctx.enter_context(tc_.tile_pool(name="kvpool", bufs=1))
        qpool = ctx.enter_context(tc_.tile_pool(name="qpool", bufs=3))
        smpool = ctx.enter_context(tc_.tile_pool(name="smpool", bufs=6))
        ppool = ctx.enter_context(tc_.tile_pool(name="ppool", bufs=2, space="PSUM"))

        # warm up the collective machinery while the front-end runs
        wut = cpool.tile([1, 64], F32)
        nc.vector.memset(wut, 0.0)
        nc.sync.dma_start(out=wuin[:, :], in_=wut)
        nc.gpsimd.collective_compute(
            "AllGather", ALU.bypass, replica_groups=[list(range(NCORES))],
            ins=[wuin[:, :]], outs=[wuout[:, :]],
        )

        # ---- constants ----
        idf = cpool.tile([128, 128], F32)
        make_identity(nc, idf)
        idb = cpool.tile([128, 128], BF16)
        make_identity(nc, idb)
        ones_bf = cpool.tile([1, 512], BF16)
        nc.vector.memset(ones_bf, 1.0)
        eps = cpool.tile([128, 1], F32)
        nc.vector.memset(eps, 1e-5)
        b3eps = cpool.tile([128, 1], F32)
        nc.vector.memset(b3eps, LN3 * 1e-5)
        d2dup = cpool.tile([128, L], F32)
        nc.sync.dma_start(out=d2dup, in_=prm["d2dup"][:, :])
        pedup = cpool.tile([128, D], F32)
        nc.sync.dma_start(out=pedup, in_=prm["pedup"][:, :])

        def bc_tile(pool, name, width, dt=F32, tag=None):
            t = pool.tile([128, width], dt, tag=tag or f"bc_{name}")
            src = prm[name].ap()
            bcast = bass.AP(tensor=src.tensor, offset=src.offset,
                            ap=[[0, 128]] + src.ap)
            nc.sync.dma_start(out=t, in_=bcast)
            return t

        def ln_inplace(x_sl, g_bc, b_bc):
            stats = smpool.tile([128, 6], F32, tag="lnstats")
            nc.vector.bn_stats(out=stats, in_=x_sl)
            mv = smpool.tile([128, 2], F32, tag="lnmv")
            nc.vector.bn_aggr(out=mv, in_=stats)
            rstd = smpool.tile([128, 1], F32, tag="lnrstd")
            nc.scalar.activation(out=rstd, in_=mv[:, 1:2], func=ACTF.Sqrt,
                                 bias=eps, scale=1.0)
            nc.vector.reciprocal(out=rstd, in_=rstd)
            nc.vector.tensor_scalar(out=x_sl, in0=x_sl, scalar1=mv[:, 0:1],
                                    scalar2=rstd, op0=ALU.subtract, op1=ALU.mult)
            nc.vector.tensor_tensor(out=x_sl, in0=x_sl, in1=g_bc, op=ALU.mult)
            nc.vector.tensor_tensor(out=x_sl, in0=x_sl, in1=b_bc, op=ALU.add)

        def transpose_to_bf16(src_f32, dst_bf):
            # src [128, TC, 512] f32 -> dst [128, DC, TOK] bf16 (transposed)
            for tcc in range(TC):
                for dc in range(DC):
                    pt = ppool.tile([128, 128], F32, tag="trf")
                    nc.tensor.transpose(pt, src_f32[:, tcc, dc * 128:(dc + 1) * 128], idf)
                    nc.vector.tensor_copy(out=dst_bf[:, dc, tcc * 128:(tcc + 1) * 128], in_=pt)

        # ---------------- front-end (replicated) ----------------
        alphas = cpool.tile([128, B], F32)
        X = xpool.tile([128, TC, D], F32, tag="X")
        fe_ctx = contextlib.ExitStack()
        with fe_ctx:
            fpool = fe_ctx.enter_context(tc_.tile_pool(name="fpool", bufs=1))
            f2pool = fe_ctx.enter_context(tc_.tile_pool(name="f2pool", bufs=2))

            few = {}
            for n in ["lfc1_w", "lfc2_w", "gq_w", "gk_w", "gfc_w", "ffc_w", "ofc_w"]:
                t = fpool.tile(list(prm[n].shape), F32, tag=n)
                nc.sync.dma_start(out=t, in_=prm[n][:, :])
                few[n] = t
            feb = {}
            for n in ["lfc1_b", "lfc2_b", "gq_b", "gk_b", "gfc_b", "ffc_b"]:
                t = fpool.tile([HID, 1], F32, tag=n)
                nc.sync.dma_start(out=t, in_=prm[n].ap()[:, None])
                feb[n] = t
            ofcb = fpool.tile([128, 1], F32)
            src = prm["ofc_b"].ap()
            nc.sync.dma_start(out=ofcb, in_=bass.AP(tensor=src.tensor, offset=src.offset,
                                                    ap=[[0, 128]] + src.ap))
            gfcb_bc = bc_tile(fpool, "gfc_b", HID, tag="gfcb_bc")
            hng_bc = bc_tile(fpool, "hn_g", HID)
            hnb_bc = bc_tile(fpool, "hn_b", HID)
            neT = fpool.tile([HID, C], F32)
            nc.sync.dma_start(out=neT, in_=prm["node_embT"][:, :])
            eye55 = fpool.tile([C, C], F32)
            nc.sync.dma_start(out=eye55, in_=prm["eye55"][:, :])
            tokw_a = fpool.tile([128, D], F32)
            nc.sync.dma_start(out=tokw_a, in_=prm["tokw"][0:128, :])
            tokw_b = fpool.tile([64, D], F32)
            nc.sync.dma_start(out=tokw_b, in_=prm["tokw"][128:192, :])

            # simp = 0.2*relu(ne@neT) + eye
            pt = ppool.tile([128, 512], F32, tag="mm")
            nc.tensor.matmul(pt[:C, :C], neT, neT, start=True, stop=True)
            simp = fpool.tile([C, C], F32)
            nc.scalar.activation(out=simp, in_=pt[:C, :C], func=ACTF.Relu)
            nc.vector.tensor_scalar_mul(out=simp, in0=simp, scalar1=0.2)
            nc.vector.tensor_tensor(out=simp, in0=simp, in1=eye55, op=ALU.add)

            def fe_mm(lhsT, rhs, m, n, func=None, bias=None, tag="fe"):
                ptm = ppool.tile([128, 512], F32, tag="mm")
                nc.tensor.matmul(ptm[:m, :n], lhsT, rhs, start=True, stop=True)
                ot = f2pool.tile([m, n], F32, tag=f"fe_{tag}")
                if func is None:
                    nc.vector.tensor_copy(out=ot, in_=ptm[:m, :n])
                else:
                    nc.scalar.activation(out=ot, in_=ptm[:m, :n], func=func,
                                         bias=bias if bias is not None else 0.0)
                return ot

            def fe_tr(in_, m, n, poff=0, tag="tr"):
                # in_ [m, n] at base partition poff -> out tile [n, m] at base 0
                ptm = ppool.tile([128, 512], F32, tag="mm")
                nc.tensor.transpose(ptm[:n, :m], in_, idf[poff:poff + m, poff:poff + m])
                ot = f2pool.tile([n, m], F32, tag=f"fetr_{tag}")
                nc.vector.tensor_copy(out=ot, in_=ptm[:n, :m])
                return ot

            for bp in range(2):
                xp = f2pool.tile([128, L], F32, tag="xp")
                nc.vector.memset(xp, 0.0)
                for half in range(2):
                    b = 2 * bp + half
                    nc.sync.dma_start(out=xp[64 * half:64 * half + C, :], in_=prm["xT"][b])
                acc = f2pool.tile([128, L], F32, tag="acc")
                nc.vector.tensor_tensor(out=acc[:, 1:], in0=xp[:, 1:], in1=xp[:, :L - 1], op=ALU.add)
                nc.vector.tensor_tensor(out=acc[:, 0:1], in0=xp[:, 0:1], in1=xp[:, 0:1], op=ALU.add)
                nc.vector.tensor_tensor(out=acc[:, :L - 1], in0=acc[:, :L - 1], in1=xp[:, 1:], op=ALU.add)
                nc.vector.tensor_tensor(out=acc[:, L - 1:], in0=acc[:, L - 1:], in1=xp[:, L - 1:], op=ALU.add)
                nc.vector.tensor_tensor(out=acc[:, 2:], in0=acc[:, 2:], in1=xp[:, :L - 2], op=ALU.add)
                nc.vector.tensor_tensor(out=acc[:, 0:2], in0=acc[:, 0:2],
                                        in1=xp[:, 0:1].to_broadcast([128, 2]), op=ALU.add)
                nc.vector.tensor_tensor(out=acc[:, :L - 2], in0=acc[:, :L - 2], in1=xp[:, 2:], op=ALU.add)
                nc.vector.tensor_tensor(out=acc[:, L - 2:], in0=acc[:, L - 2:],
                                        in1=xp[:, L - 1:].to_broadcast([128, 2]), op=ALU.add)
                res = f2pool.tile([128, L], F32, tag="res")
                nc.vector.tensor_scalar_mul(out=res, in0=acc, scalar1=0.2)
                nc.vector.tensor_tensor(out=res, in0=xp, in1=res, op=ALU.subtract)
                a = f2pool.tile([128, L], F32, tag="absres")
                nc.scalar.activation(out=a, in_=res, func=ACTF.Abs)
                stats = smpool.tile([128, 6], F32, tag="festats")
                nc.vector.bn_stats(out=stats, in_=a)
                mv = smpool.tile([128, 2], F32, tag="femv")
                nc.vector.bn_aggr(out=mv, in_=stats)
                mx = smpool.tile([128, 1], F32, tag="femx")
                nc.vector.tensor_reduce(out=mx, in_=a, axis=AX.X, op=ALU.max)
                h0 = f2pool.tile([128, 4], F32, tag="h0")
                nc.gpsimd.tensor_copy(out=h0[:, 0:1], in_=mv[:, 0:1])
                nc.gpsimd.tensor_copy(out=h0[:, 1:2], in_=mx)
                nc.scalar.activation(out=h0[:, 2:3], in_=mv[:, 1:2], func=ACTF.Sqrt)
                m1sq = smpool.tile([128, 1], F32, tag="fem1")
                nc.scalar.activation(out=m1sq, in_=mv[:, 0:1], func=ACTF.Square)
                nc.vector.tensor_tensor(out=h0[:, 3:4], in0=mv[:, 1:2], in1=m1sq, op=ALU.add)

                for half in range(2):
                    b = 2 * bp + half
                    off = 64 * half
                    h0T = fe_tr(h0[off:off + C, :], C, 4, poff=off, tag="h0T")  # [4, 55]
                    z1T = fe_mm(few["lfc1_w"], h0T, HID, C, ACTF.Relu, feb["lfc1_b"], tag="z1T")
                    hlocT = fe_mm(few["lfc2_w"], z1T, HID, C, ACTF.Identity, feb["lfc2_b"], tag="hlocT")
                    qT = fe_mm(few["gq_w"], h0T, HID, C, ACTF.Identity, feb["gq_b"], tag="qT")
                    kT = fe_mm(few["gk_w"], h0T, HID, C, ACTF.Identity, feb["gk_b"], tag="kT")
                    hgT = fe_mm(few["gfc_w"], h0T, HID, C, ACTF.Identity, feb["gfc_b"], tag="hgT")
                    hg = fe_mm(h0T, few["gfc_w"], C, HID, tag="hg")
                    nc.vector.tensor_tensor(out=hg, in0=hg, in1=gfcb_bc[:C, :HID], op=ALU.add)
                    Ain = fe_mm(qT, kT, C, C, ACTF.Relu, tag="Ain")
                    nc.vector.tensor_tensor(out=Ain, in0=Ain, in1=simp, op=ALU.add)
                    rsum = smpool.tile([C, 1], F32, tag="fersum")
                    A = f2pool.tile([C, C], F32, tag="A")
                    nc.scalar.activation(out=A, in_=Ain, func=ACTF.Exp, accum_out=rsum)
                    nc.vector.reciprocal(out=rsum, in_=rsum)
                    nc.vector.tensor_scalar_mul(out=A, in0=A, scalar1=rsum)
                    AT = fe_tr(A, C, C, tag="AT")
                    AhgT = fe_mm(hg, AT, HID, C, ACTF.Relu, tag="AhgT")
                    nc.vector.tensor_scalar_mul(out=AhgT, in0=AhgT, scalar1=0.3)
                    nc.vector.tensor_tensor(out=AhgT, in0=AhgT, in1=hgT, op=ALU.add)
                    nc.vector.tensor_scalar_mul(out=AhgT, in0=AhgT, scalar1=0.1)
                    nc.vector.tensor_tensor(out=AhgT, in0=AhgT, in1=hlocT, op=ALU.add)
                    fuse = fe_tr(AhgT, HID, C, tag="fuse")                      # [55, 16]
                    st2 = smpool.tile([C, 6], F32, tag="fest2")
                    nc.vector.bn_stats(out=st2, in_=fuse)
                    mv2 = smpool.tile([C, 2], F32, tag="femv2")
                    nc.vector.bn_aggr(out=mv2, in_=st2)
                    rstd = smpool.tile([C, 1], F32, tag="ferstd")
                    nc.scalar.activation(out=rstd, in_=mv2[:, 1:2], func=ACTF.Sqrt,
                                         bias=eps[:C], scale=1.0)
                    nc.vector.reciprocal(out=rstd, in_=rstd)
                    nc.vector.tensor_scalar(out=fuse, in0=fuse, scalar1=mv2[:, 0:1],
                                            scalar2=rstd, op0=ALU.subtract, op1=ALU.mult)
                    nc.vector.tensor_tensor(out=fuse, in0=fuse, in1=hng_bc[:C, :], op=ALU.mult)
                    nc.vector.tensor_tensor(out=fuse, in0=fuse, in1=hnb_bc[:C, :], op=ALU.add)
                    hnT = fe_tr(fuse, C, HID, tag="hnT")                       # [16, 55]
                    hT = fe_mm(few["ffc_w"], hnT, HID, C, ACTF.Relu, feb["ffc_b"], tag="hT")
                    pal = ppool.tile([128, 512], F32, tag="mm")
                    nc.tensor.matmul(pal[:C, :1], hT, few["ofc_w"], start=True, stop=True)
                    alf = smpool.tile([C, 1], F32, tag="fealf")
                    nc.scalar.activation(out=alf, in_=pal[:C, :1], func=ACTF.Tanh,
                                         bias=ofcb[:C], scale=1.0)
                    nc.vector.tensor_scalar(out=alphas[:C, b:b + 1], in0=alf, scalar1=0.3,
                                            scalar2=1.2, op0=ALU.mult, op1=ALU.add)

            # windowed enhancement + circular-conv token embedding -> X
            for b in range(B):
                xw = f2pool.tile([C, 5, LS + 2], F32, tag="xw")
                nc.sync.dma_start(out=xw, in_=prm["xw5T"][b].rearrange("t c w -> c t w"))
                bgw = f2pool.tile([C, LS + 2], F32, tag="bgw")
                nc.vector.tensor_tensor(out=bgw, in0=xw[:, 0, :], in1=xw[:, 1, :], op=ALU.add)
                nc.vector.tensor_tensor(out=bgw, in0=bgw, in1=xw[:, 2, :], op=ALU.add)
                nc.vector.tensor_tensor(out=bgw, in0=bgw, in1=xw[:, 3, :], op=ALU.add)
                nc.vector.tensor_tensor(out=bgw, in0=bgw, in1=xw[:, 4, :], op=ALU.add)
                nc.vector.tensor_scalar_mul(out=bgw, in0=bgw, scalar1=0.2)
                nc.vector.tensor_tensor(out=bgw, in0=xw[:, 2, :], in1=bgw, op=ALU.subtract)
                nc.vector.tensor_scalar_mul(out=bgw, in0=bgw, scalar1=alphas[:C, b:b + 1])
                enh = f2pool.tile([C, LS + 2], F32, tag="enh")
                nc.vector.tensor_tensor(out=enh, in0=xw[:, 2, :], in1=bgw, op=ALU.add)
                x3a = f2pool.tile([128, LS], F32, tag="x3a")
                nc.vector.memset(x3a, 0.0)
                x3b = f2pool.tile([64, LS], F32, tag="x3b")
                nc.vector.memset(x3b, 0.0)
                nc.gpsimd.tensor_copy(out=x3a[0:C, :], in_=enh[:, 0:LS])
                nc.gpsimd.tensor_copy(out=x3a[64:64 + C, :], in_=enh[:, 1:LS + 1])
                nc.gpsimd.tensor_copy(out=x3b[0:C, :], in_=enh[:, 2:LS + 2])
                pt0 = ppool.tile([128, 512], F32, tag="mm")
                nc.tensor.matmul(pt0[:LS, :], x3a, tokw_a, start=True, stop=False)
                nc.tensor.matmul(pt0[:LS, :], x3b, tokw_b, start=False, stop=True)
                off = 64 * (b % 2)
                nc.vector.tensor_tensor(out=X[off:off + LS, b // 2, :], in0=pt0[:LS, :],
                                        in1=pedup[off:off + LS, :], op=ALU.add)

        # debug taps: X post-embedding, alphas
        nc.sync.dma_start(out=outs["o_dbg"][0], in_=X[:, 0, :])
        nc.sync.dma_start(out=outs["o_dbg"][1], in_=X[:, 1, :])
        nc.sync.dma_start(out=outs["o_dbg"][2][:, 0:B], in_=alphas)

        # ---------------- transformer layers ----------------
        for li in range(NL):
            w = {}
            for n in ["wq", "wk", "wv", "wo", "c1", "c2"]:
                t = wpool.tile([128, DC, D], BF16, tag=n)
                nc.sync.dma_start(out=t, in_=prm[f"{n}{li}"].ap().rearrange("(dc p) n -> p dc n", p=128))
                w[n] = t
            wsig = wpool.tile([128, DC, H], BF16, tag="wsig")
            nc.sync.dma_start(out=wsig, in_=prm[f"wsig{li}"].ap().rearrange("(dc p) n -> p dc n", p=128))
            brow = {}
            for n in ["bq", "bk", "bv", "bo", "c1b", "c2b", "bsig"]:
                t = lnpool.tile([1, D if n != "bsig" else H], BF16, tag=f"br_{n}")
                nc.sync.dma_start(out=t, in_=prm[f"{n}{li}"][:, :])
                brow[n] = t
            lnbc = {}
            for n in ["ln1g", "ln1b", "ln2g", "ln2b"]:
                lnbc[n] = bc_tile(lnpool, f"{n}{li}", D, F32, tag=f"bc_{n}")

            XT = s2pool.tile([128, DC, TOK], BF16, tag="XT")
            transpose_to_bf16(X, XT)

            # projections on own tokens
            QT = s2pool.tile([128, DC, TOK], BF16, tag="QT")
            KTo = s2pool.tile([128, DC, TOK], BF16, tag="KTo")
            for name, dst, bn in [("wq", QT, "bq"), ("wk", KTo, "bk")]:
                for dco in range(DC):
                    pt1 = ppool.tile([128, 512], F32, tag="mm")
                    for dci in range(DC):
                        nc.tensor.matmul(pt1[:, :TOK], w[name][:, dci, dco * 128:(dco + 1) * 128],
                                         XT[:, dci, :], start=(dci == 0), stop=False)
                    nc.tensor.matmul(pt1[:, :TOK], brow[bn][:, dco * 128:(dco + 1) * 128],
                                     ones_bf[:, :TOK], start=False, stop=True)
                    nc.vector.tensor_copy(out=dst[:, dco, :], in_=pt1[:, :TOK])
            Vo = s2pool.tile([128, TC, D], BF16, tag="Vo")
            for tcc in range(TC):
                pt2 = ppool.tile([128, 512], F32, tag="mm")
                for dci in range(DC):
                    nc.tensor.matmul(pt2, XT[:, dci, tcc * 128:(tcc + 1) * 128], w["wv"][:, dci, :],
                                     start=(dci == 0), stop=False)
                nc.tensor.matmul(pt2, ones_bf[:, tcc * 128:(tcc + 1) * 128], brow["bv"],
                                 start=False, stop=True)
                nc.vector.tensor_copy(out=Vo[:, tcc, :], in_=pt2)

            # ship own K/V and all-gather (rank r rows [1024r, 1024r+1024) of agout)
            nc.sync.dma_start(out=agin[0:512, :].rearrange("(dc p) t -> p dc t", p=128), in_=KTo)
            nc.sync.dma_start(out=agin[512:1024, :].rearrange("(p t j) c -> p t (j c)", p=128, t=2),
                              in_=Vo)
            nc.gpsimd.collective_compute(
                "AllGather", ALU.bypass,
                replica_groups=[list(range(NCORES))],
                ins=[agin[:, :]], outs=[agout[:, :]],
            )

            # sigma / prior / sigma-out (independent of the collective)
            sig_t = s2pool.tile([128, TC, H], F32, tag="sig")
            for tcc in range(TC):
                pt3 = ppool.tile([128, 512], F32, tag="mm")
                for dci in range(DC):
                    nc.tensor.matmul(pt3[:, :H], XT[:, dci, tcc * 128:(tcc + 1) * 128],
                                     wsig[:, dci, :], start=(dci == 0), stop=False)
                nc.tensor.matmul(pt3[:, :H], ones_bf[:, tcc * 128:(tcc + 1) * 128], brow["bsig"],
                                 start=False, stop=True)
                nc.scalar.activation(out=sig_t[:, tcc, :], in_=pt3[:, :H], func=ACTF.Sigmoid, scale=5.0)
                nc.scalar.activation(out=sig_t[:, tcc, :], in_=sig_t[:, tcc, :], func=ACTF.Exp,
                                     scale=LN3, bias=b3eps)
                nc.vector.tensor_scalar(out=sig_t[:, tcc, :], in0=sig_t[:, tcc, :], scalar1=1.0,
                                        scalar2=1.0, op0=ALU.subtract, op1=ALU.mult)
            if li == 0:
                nc.sync.dma_start(out=outs["o_dbg"][3][:, 0:TC * H],
                                  in_=sig_t.rearrange("p a b -> p (a b)"))
            s1 = s2pool.tile([128, TC * H], F32, tag="s1")
            ccn = s2pool.tile([128, TC * H], F32, tag="ccn")
            sflat = sig_t.rearrange("p a b -> p (a b)")
            nc.scalar.activation(out=s1, in_=sflat, func=ACTF.Square)
            nc.vector.reciprocal(out=s1, in_=s1)
            nc.vector.tensor_scalar_mul(out=s1, in0=s1, scalar1=-0.5)
            nc.scalar.activation(out=ccn, in_=sflat, func=ACTF.Ln, scale=math.sqrt(2.0 * math.pi))
            nc.vector.tensor_scalar_mul(out=ccn, in0=ccn, scalar1=-1.0)
            # gathered K/V into SBUF
            KT = kvpool.tile([128, DC, B * L], BF16, tag="KT")      # cols r*256+b*64+l'
            for r in range(NCORES):
                nc.sync.dma_start(
                    out=KT[:, :, r * 256:(r + 1) * 256],
                    in_=agout[1024 * r:1024 * r + 512, :].rearrange("(dc p) t -> p dc t", p=128))
            V = kvpool.tile([128, B * 4, D], BF16, tag="V")         # [s-part, (b,sc), d]
            for r in range(NCORES):
                src = agout[1024 * r + 512:1024 * (r + 1), :].rearrange(
                    "(h2 l t j) c -> l t h2 (j c)", h2=2, l=64, t=2, j=2)
                dst = V[64 * (r % 2):64 * (r % 2) + 64].rearrange(
                    "p (t h2 rr) d -> p t h2 rr d", t=2, h2=2)[:, :, :, r // 2, :]
                nc.sync.dma_start(out=dst, in_=src)

            # attention per (h, tc)
            newxT = s2pool.tile([128, DC, TOK], BF16, tag="newxT")
            for h in range(H):
                hp = 64 * (h % 2)
                hc = h // 2
                for tcc in range(TC):
                    col = tcc * H + h
                    pr = qpool.tile([128, L], F32, tag="prsg")
                    nc.scalar.activation(out=pr, in_=d2dup, func=ACTF.Exp,
                                         scale=s1[:, col:col + 1], bias=ccn[:, col:col + 1])
                    nc.sync.dma_start(out=outs["o_prior"][li, h, tcc], in_=pr)
                    sg = qpool.tile([128, L], F32, tag="prsg")
                    nc.vector.tensor_scalar(out=sg, in0=d2dup, scalar1=0.0,
                                            scalar2=sig_t[:, tcc, h:h + 1],
                                            op0=ALU.mult, op1=ALU.add)
                    nc.sync.dma_start(out=outs["o_sigma"][li, h, tcc], in_=sg)
                    sc_ps = ppool.tile([128, 512], F32, tag="mm")
                    for half in range(2):
                        b = 2 * tcc + half
                        rhs = KT[hp:hp + 64, hc].rearrange("p (r q) -> p r q", q=256)[:, :, b * 64:b * 64 + 64]
                        nc.tensor.matmul(sc_ps[64 * half:64 * half + 64, :],
                                         QT[hp:hp + 64, hc,
                                            tcc * 128 + 64 * half:tcc * 128 + 64 * half + 64],
                                         rhs, start=True, stop=True)
                    Praw = qpool.tile([128, L], F32, tag="Praw")
                    rsum = smpool.tile([128, 1], F32, tag="rsum")
                    nc.scalar.activation(out=Praw, in_=sc_ps, func=ACTF.Exp, scale=SCALE,
                                         accum_out=rsum)
                    nc.vector.reciprocal(out=rsum, in_=rsum)
                    ser = qpool.tile([128, L], F32, tag="ser")
                    nc.vector.tensor_scalar_mul(out=ser, in0=Praw, scalar1=rsum)
                    nc.sync.dma_start(out=outs["o_series"][li, h, tcc], in_=ser)
                    Pn = qpool.tile([128, L], BF16, tag="Pn")
                    nc.vector.tensor_scalar_mul(out=Pn, in0=Praw, scalar1=rsum)
                    PT = qpool.tile([128, 4, 128], BF16, tag="PT")
                    ptb = ppool.tile([128, 4, 128], BF16, tag="trb")
                    for sc in range(4):
                        nc.tensor.transpose(ptb[:, sc, :], Pn[:, sc * 128:(sc + 1) * 128], idb)
                    nc.scalar.copy(out=PT, in_=ptb)
                    for half in range(2):
                        b = 2 * tcc + half
                        av = ppool.tile([64, 64], F32, tag="av")
                        for sc in range(4):
                            nc.tensor.matmul(av, V[:, 4 * b + sc, 64 * h:64 * h + 64],
                                             PT[:, sc, 64 * half:64 * half + 64],
                                             start=(sc == 0), stop=(sc == 3))
                        nc.vector.tensor_copy(
                            out=newxT[hp:hp + 64, hc,
                                      tcc * 128 + 64 * half:tcc * 128 + 64 * half + 64],
                            in_=av)

            # wo + residual + LN1
            xr = s2pool.tile([128, TC, D], F32, tag="xr")
            for tcc in range(TC):
                pt4 = ppool.tile([128, 512], F32, tag="mm")
                for dci in range(DC):
                    nc.tensor.matmul(pt4, newxT[:, dci, tcc * 128:(tcc + 1) * 128], w["wo"][:, dci, :],
                                     start=(dci == 0), stop=False)
                nc.tensor.matmul(pt4, ones_bf[:, tcc * 128:(tcc + 1) * 128], brow["bo"],
                                 start=False, stop=True)
                nc.vector.tensor_tensor(out=xr[:, tcc, :], in0=pt4, in1=X[:, tcc, :], op=ALU.add)
                ln_inplace(xr[:, tcc, :], lnbc["ln1g"], lnbc["ln1b"])

            # FFN + LN2 (stage 1 in transposed dff-major layout)
            xrT = s2pool.tile([128, DC, TOK], BF16, tag="xrT")
            transpose_to_bf16(xr, xrT)
            y1T = s2pool.tile([128, DC, TOK], BF16, tag="y1T")
            for dfo in range(DC):
                pt5 = ppool.tile([128, 512], F32, tag="mm")
                for dci in range(DC):
                    nc.tensor.matmul(pt5[:, :TOK], w["c1"][:, dci, dfo * 128:(dfo + 1) * 128],
                                     xrT[:, dci, :], start=(dci == 0), stop=False)
                nc.tensor.matmul(pt5[:, :TOK], brow["c1b"][:, dfo * 128:(dfo + 1) * 128],
                                 ones_bf[:, :TOK], start=False, stop=True)
                tg = qpool.tile([128, TOK], F32, tag="tg")
                nc.scalar.activation(out=tg, in_=pt5[:, :TOK], func=ACTF.Erf,
                                     scale=1.0 / math.sqrt(2.0))
                nc.vector.tensor_scalar(out=tg, in0=tg, scalar1=1.0, scalar2=0.5,
                                        op0=ALU.add, op1=ALU.mult)
                nc.vector.tensor_tensor(out=y1T[:, dfo, :], in0=tg, in1=pt5[:, :TOK],
                                        op=ALU.mult)
            Xn = xpool.tile([128, TC, D], F32, tag="X")
            for tcc in range(TC):
                pt6 = ppool.tile([128, 512], F32, tag="mm")
                for dci in range(DC):
                    nc.tensor.matmul(pt6, y1T[:, dci, tcc * 128:(tcc + 1) * 128], w["c2"][:, dci, :],
                                     start=(dci == 0), stop=False)
                nc.tensor.matmul(pt6, ones_bf[:, tcc * 128:(tcc + 1) * 128], brow["c2b"],
                                 start=False, stop=True)
                nc.vector.tensor_tensor(out=Xn[:, tcc, :], in0=pt6, in1=xr[:, tcc, :], op=ALU.add)
                ln_inplace(Xn[:, tcc, :], lnbc["ln2g"], lnbc["ln2b"])
            X = Xn

        # ---------------- final projection ----------------
        lnfg_bc = bc_tile(cpool, "lnfg", D)
        lnfb_bc = bc_tile(cpool, "lnfb", D)
        projw = cpool.tile([128, DC, C], BF16)
        nc.sync.dma_start(out=projw, in_=prm["projw"].ap().rearrange("(dc p) n -> p dc n", p=128))
        projb = cpool.tile([1, C], BF16)
        nc.sync.dma_start(out=projb, in_=prm["projb"][:, :])
        for tcc in range(TC):
            ln_inplace(X[:, tcc, :], lnfg_bc, lnfb_bc)
        XfT = s2pool.tile([128, DC, TOK], BF16, tag="XfT")
        transpose_to_bf16(X, XfT)
        for tcc in range(TC):
            pt7 = ppool.tile([128, 512], F32, tag="mm")
            for dci in range(DC):
                nc.tensor.matmul(pt7[:, :C], XfT[:, dci, tcc * 128:(tcc + 1) * 128],
                                 projw[:, dci, :], start=(dci == 0), stop=False)
            nc.tensor.matmul(pt7[:, :C], ones_bf[:, tcc * 128:(tcc + 1) * 128], projb,
                             start=False, stop=True)
            fo = qpool.tile([128, C], F32, tag="fo")
            nc.vector.tensor_copy(out=fo, in_=pt7[:, :C])
            nc.sync.dma_start(out=outs["o_final"][tcc], in_=fo)


# ======================= host side =======================

_NC = None


def _get_nc():
    global _NC
    if _NC is None:
        _NC = build()
    return _NC


def _host_inputs(x, params):
    """Build the replicated + per-core input maps."""
    p = params
    rep = {}
    rep["xT"] = np.ascontiguousarray(np.asarray(x, np.float32).transpose(0, 2, 1))

    rep["node_embT"] = np.ascontiguousarray(np.asarray(p["node_emb"], np.float32).T)
    rep["eye55"] = np.eye(C, dtype=np.float32)
    for n in ["lfc1_w", "lfc2_w", "gq_w", "gk_w", "gfc_w", "ffc_w", "ofc_w",
              "lfc1_b", "lfc2_b", "gq_b", "gk_b", "gfc_b", "ffc_b", "hn_g", "hn_b", "ofc_b"]:
        rep[n] = np.ascontiguousarray(np.asarray(p[n], np.float32))

    tokw = np.zeros((192, D), np.float32)
    tw = np.asarray(p["tok_w"], np.float32)            # [3, C, D] (WIO)
    for wtap in range(3):
        tokw[64 * wtap:64 * wtap + C] = tw[wtap]
    rep["tokw"] = tokw

    for i, lp in enumerate(p["layers"]):
        for n, k in [("wq", "wq"), ("wk", "wk"), ("wv", "wv"), ("wo", "wo"),
                     ("c1", "c1_w"), ("c2", "c2_w")]:
            rep[f"{n}{i}"] = np.asarray(lp[k], np.float32).astype(BF)
        rep[f"wsig{i}"] = np.asarray(lp["wsig"], np.float32).astype(BF)
        for n, k in [("bq", "bq"), ("bk", "bk"), ("bv", "bv"), ("bo", "bo"),
                     ("c1b", "c1_b"), ("c2b", "c2_b")]:
            rep[f"{n}{i}"] = np.asarray(lp[k], np.float32).astype(BF).reshape(1, -1)
        rep[f"bsig{i}"] = np.asarray(lp["bsig"], np.float32).astype(BF).reshape(1, -1)
        for n, k in [("ln1g", "ln1_g"), ("ln1b", "ln1_b"), ("ln2g", "ln2_g"), ("ln2b", "ln2_b")]:
            rep[f"{n}{i}"] = np.asarray(lp[k], np.float32)
    rep["lnfg"] = np.asarray(p["lnf_g"], np.float32)
    rep["lnfb"] = np.asarray(p["lnf_b"], np.float32)
    rep["projw"] = np.asarray(p["proj_w"], np.float32).astype(BF)
    rep["projb"] = np.asarray(p["proj_b"], np.float32).astype(BF).reshape(1, -1)

    # positional embedding rows (constant, replicated formula)
    pos = np.arange(L, dtype=np.float32)[:, None]
    div = np.exp(np.arange(0, D, 2, dtype=np.float32) * (-math.log(10000.0) / D))
    pe = np.zeros((L, D), np.float32)
    pe[:, 0::2] = np.sin(pos * div)
    pe[:, 1::2] = np.cos(pos * div)
    dist = np.abs(np.arange(L, dtype=np.float32)[:, None] - np.arange(L, dtype=np.float32)[None, :])
    d2 = dist.astype(np.float32) ** 2

    xf = np.asarray(x, np.float32)
    in_maps = []
    for i in range(NCORES):
        m = dict(rep)
        rows = np.arange(LS * i, LS * i + LS)
        m["pedup"] = np.ascontiguousarray(
            np.concatenate([pe[rows], pe[rows]], axis=0))
        m["d2dup"] = np.ascontiguousarray(
            np.concatenate([d2[rows], d2[rows]], axis=0))
        # window: positions j=0..65 are l = 64i-1+j (mod L); 5 clamped bg taps
        win = (np.arange(LS * i - 1, LS * i + LS + 1)) % L      # [66]
        xw5 = np.empty((B, 5, C, LS + 2), np.float32)
        for t in range(5):
            idx = np.clip(win + (t - 2), 0, L - 1)
            xw5[:, t] = xf[:, idx, :].transpose(0, 2, 1)
        m["xw5T"] = np.ascontiguousarray(xw5)
        in_maps.append(m)
    return in_maps


def kernel(x, params):
    from concourse.bass_utils import run_bass_kernel_spmd

    nc = _get_nc()
    in_maps = _host_inputs(x, params)
    res = run_bass_kernel_spmd(nc, in_maps, list(range(NCORES)))
    results = res.results

    series = np.empty((NL, B, H, L, L), np.float32)
    prior = np.empty((NL, B, H, L, L), np.float32)
    sigma = np.empty((NL, B, H, L, L), np.float32)
    out = np.empty((B, L, C), np.float32)
    for i in range(NCORES):
        r = results[i]
        # o_* [NL, H, TC, 128, L]; partition p = 64*(b%2)+l', tc = b//2
        for name, dstf in [("o_series", series), ("o_prior", prior), ("o_sigma", sigma)]:
            v = r[name]
            for b in range(B):
                sl = v[:, :, b // 2, 64 * (b % 2):64 * (b % 2) + 64, :]     # [NL,H,64,L]
                dstf[:, b, :, LS * i:LS * i + LS, :] = sl
        vf = r["o_final"]
        for b in range(B):
            out[b, LS * i:LS * i + LS, :] = vf[b // 2, 64 * (b % 2):64 * (b % 2) + 64, :]
    return (out, series, prior, sigma)
